# revision 7
# baseline (speedup 1.0000x reference)
"""Trainium2 Bass kernel for nn_DynamicFeatureGroupingLayer.

Reference computation (B=4096, G=10 groups of S=100 features, M=4 masks,
H=512 hidden):
    mask = entmax(1.1, W_masks)                       # [G,M,S]
    h_t[b,g,m,:] = (x_g[b] * mask[g,m]) @ W_t[g].T    # t in {1,2}
    n_t = layernorm(h_t) * ln_w + ln_b
    out[b,g] = sum_m relu(sigmoid(n_1) * n_2)         # [B, G*H]

Strategy:
  * Data-parallel over batch across 8 cores (512 rows each).
  * Host folds the mask into the weights: W~_t[g,m] = mask[g,m,:] * W_t[g]
    so h_t = x_g @ W~_t.T is a plain matmul (K=S=100, stationary = x chunk).
  * LN means come free as extra matmul columns (sum_h W~/H).
  * LN second moments via a Cholesky-Gram trick: ss = ||L^T x||^2 with
    L = chol(W~^T W~) [S,S]; computing stats in x-space (S=100) instead of
    h-space (H=512) cuts the stats passes ~4x.
  * Epilogue per (group, mask): sigmoid as one ACT pass with per-partition
    scale/bias (rs1, -mu1*rs1); gate multiply + relu as two DVE passes using
    relu(s*(h2-mu2))*rs2 == sigmoid(n1)*relu(n2) (s, rs2 > 0); mask-sum
    accumulation on GPSIMD.
"""

import numpy as np

B = 4096
INPUT_SIZE = 1000
H = 512
M = 4
S = 100
G = 10
N_CORES = 8
BC = B // N_CORES            # batch rows per core (512)
NBC = BC // 128              # 128-row chunks per core (4)
EPS_LN = 1e-5

# matmul dtype for the PE inputs: "f32r" (full-rate, ~tf32 numerics),
# "f32" (exact, 4x slower), "bf16"
MM_DTYPE = "f32r"
# engine for the mask-sum accumulation: "gpsimd" or "vector"
ACC_ENGINE = "gpsimd"

_STATE = {}


# --------------------------------------------------------------------------
# host-side preprocessing
# --------------------------------------------------------------------------

def _entmax(alpha, v):
    v = v - np.max(v, axis=-1, keepdims=True)
    e = np.exp(v)
    s = (np.sum(e ** alpha, axis=-1, keepdims=True) + 1e-5) ** (1.0 / alpha)
    return e / s


def _host_prep(x, W_masks, W1, W2):
    """Returns (xt_per_core, W_rhs, L_rhs, MU_rhs) as float32 arrays."""
    x = np.asarray(x, np.float32)
    mask = _entmax(1.1, np.asarray(W_masks, np.float64)).astype(np.float64)
    W1 = np.asarray(W1, np.float64)
    W2 = np.asarray(W2, np.float64)

    # W~_t[g,m,h,s] = mask[g,m,s] * W_t[g,h,s]
    Wt1 = mask[:, :, None, :] * W1[:, None, :, :]        # [G,M,H,S]
    Wt2 = mask[:, :, None, :] * W2[:, None, :, :]
    # main rhs: [G, S, M*2*H], col = m*1024 + t*512 + h
    W_rhs = np.stack([Wt1, Wt2], axis=2)                  # [G,M,2,H,S]
    W_rhs = W_rhs.transpose(0, 4, 1, 2, 3).reshape(G, S, M * 2 * H)

    # mean cols: [G, S, 2M], col = 2m + t; value = sum_h W~/H
    MU = np.stack([Wt1.mean(axis=2), Wt2.mean(axis=2)], axis=2)  # [G,M,2,S]
    MU_rhs = MU.transpose(0, 3, 1, 2).reshape(G, S, 2 * M)

    # cholesky of gram matrices: [G, S, M*256], cols m*256+{0:100 -> L1, 128:228 -> L2}
    # the 8 mean cols ride in the spare tail of the m=0 block (cols 228:236)
    L_rhs = np.zeros((G, S, M * 256), np.float64)
    for g in range(G):
        for m in range(M):
            for t, Wt in enumerate((Wt1, Wt2)):
                Wm = Wt[g, m]                              # [H,S]
                Gm = Wm.T @ Wm                             # [S,S]
                jit = 1e-9 * np.trace(Gm) / S
                Lm = np.linalg.cholesky(Gm + jit * np.eye(S))
                L_rhs[g, :, m * 256 + 128 * t: m * 256 + 128 * t + S] = Lm
    L_rhs[:, :, 228:236] = MU_rhs

    # x transposed per core: xt[s, g*512 + b] = x[c*512+b, g*100+s]
    xt_cores = []
    for c in range(N_CORES):
        xc = x[c * BC:(c + 1) * BC]                        # [512, 1000]
        xt = np.ascontiguousarray(
            xc.reshape(BC, G, S).transpose(2, 1, 0).reshape(S, G * BC))
        xt_cores.append(xt)

    return (xt_cores, W_rhs.astype(np.float32), L_rhs.astype(np.float32),
            MU_rhs.astype(np.float32))


# --------------------------------------------------------------------------
# tile drain workaround (this walrus build rejects multi-wait CTRL insts)
# --------------------------------------------------------------------------

def _install_tile_patch():
    import concourse.mybir as mybir
    from concourse.tile import TileContext, ScopedClock

    if getattr(TileContext, "_drain_patched", False):
        return

    def _patched(self, tick_clock, wait_clock):
        nc = self.nc
        probe = nc.sync.nop(hint="drain_waits", nofuse=True)
        wait_clock.add_sem_waits(
            probe.ins, ScopedClock({None: tick_clock.global_clock}))
        si = probe.ins.sync_info
        if si is not None and len(si.on_wait) > 1:
            waits = list(si.on_wait)
            si.on_wait = [waits[0]]
            probe.ins.sync_info = si
            for w in waits[1:]:
                extra = nc.sync.nop(hint="drain_waits_x", nofuse=True)
                extra.ins.sync_info = mybir.SyncInfo(on_wait=[w], on_update=[])
        nc.sync.drain()
        nc.all_engine_barrier()
        popped = nc._tile_sem_poison_stack.pop()
        assert popped is self._sem_poison
        nc.clear_and_free_semaphores(list(self.sems.allocated().values()))
        nc.all_engine_barrier()

    TileContext._drain_and_barrier = _patched

    # This walrus build accepts at most ONE sync wait per instruction.
    # Split extra waits onto dedicated same-engine NOPs committed just
    # before the instruction (sequential blocking on monotonically
    # increasing semaphores is equivalent to a combined wait).
    orig_commit = TileContext._commit_instruction

    def _commit_split(self, inst, lazy_reg_writes=True):
        si = inst.sync_info
        if (
            si is not None
            and len(si.on_wait) > 1
            and inst.engine != mybir.EngineType.Unassigned
        ):
            waits = list(si.on_wait)
            for w in waits[:-1]:
                nop = mybir.InstNoOp(
                    name=self.nc.get_next_instruction_name(),
                    engine=inst.engine,
                    ins=[],
                    outs=[],
                    sync_info=mybir.SyncInfo(on_wait=[w], on_update=[]),
                )
                orig_commit(self, nop, lazy_reg_writes=False)
            si.on_wait = [waits[-1]]
            inst.sync_info = si
        return orig_commit(self, inst, lazy_reg_writes)

    TileContext._commit_instruction = _commit_split
    TileContext._drain_patched = True


# --------------------------------------------------------------------------
# device kernel
# --------------------------------------------------------------------------

def _build_program():
    import concourse.bass as bass
    import concourse.mybir as mybir
    import concourse.tile as tile

    _install_tile_patch()
    dt = mybir.dt
    AF = mybir.ActivationFunctionType
    OP = mybir.AluOpType
    mm_dt = {"f32r": dt.float32r, "f32": dt.float32, "bf16": dt.bfloat16}[MM_DTYPE]

    nc = bass.Bass()
    xt_d = nc.declare_dram_parameter("xt", [S, G * BC], mm_dt, isOutput=False)
    w_d = nc.declare_dram_parameter("w", [G, S, M * 2 * H], mm_dt, isOutput=False)
    l_d = nc.declare_dram_parameter("l", [G, S, M * 256], mm_dt, isOutput=False)
    y_d = nc.declare_dram_parameter("y", [BC, G * H], dt.float32, isOutput=True)

    with tile.TileContext(nc) as tc:
        with (
            tc.tile_pool(name="xpool", bufs=1) as xpool,
            tc.tile_pool(name="wpool", bufs=2) as wpool,
            tc.tile_pool(name="lpool", bufs=2) as lpool,
            tc.tile_pool(name="hpsum", bufs=2, space="PSUM") as hpsum,
            tc.tile_pool(name="zpsum", bufs=4, space="PSUM") as zpsum,
            tc.tile_pool(name="spool", bufs=3) as spool,
            tc.tile_pool(name="tpool", bufs=3) as tpool,
            tc.tile_pool(name="vpool", bufs=3) as vpool,
            tc.tile_pool(name="accpool", bufs=2) as accpool,
            tc.tile_pool(name="junkpool", bufs=2) as junkpool,
            tc.tile_pool(name="statpool", bufs=4) as statpool,
        ):
            xt_sb = xpool.tile([S, G * BC], mm_dt)
            nc.sync.dma_start(xt_sb[:], xt_d[:])
            eps_sb = xpool.tile([128, 1], dt.float32, tag="eps")
            nc.vector.memset(eps_sb[:], EPS_LN)

            acc_eng = nc.gpsimd if ACC_ENGINE == "gpsimd" else nc.vector

            for g in range(G):
                w_sb = wpool.tile([S, M * 2 * H], mm_dt, tag="w")
                nc.sync.dma_start(w_sb[:], w_d[g])
                l_sb = lpool.tile([S, M * 256], mm_dt, tag="l")
                nc.sync.dma_start(l_sb[:], l_d[g])

                for bc in range(NBC):
                    xch = xt_sb[:, g * BC + bc * 128: g * BC + (bc + 1) * 128]

                    # ---- matmuls: z (stats) first so they gate nothing ----
                    z_ps = []
                    for m in range(M):
                        zp = zpsum.tile([128, 256], dt.float32, tag="z")
                        nc.tensor.matmul(
                            zp[:], xch, l_sb[:, m * 256:(m + 1) * 256])
                        z_ps.append(zp)
                    h_ps = []
                    for m in range(M):
                        hp = hpsum.tile([128, 2 * H], dt.float32, tag="h")
                        nc.tensor.matmul(
                            hp[:, 0:H], xch, w_sb[:, m * 2 * H: m * 2 * H + H])
                        nc.tensor.matmul(
                            hp[:, H:2 * H], xch,
                            w_sb[:, m * 2 * H + H: (m + 1) * 2 * H])
                        h_ps.append(hp)

                    # ---- stats: ss[:, 2m+t] = sum_j z_t^2 (ACT square+accum) ----
                    ss = statpool.tile([128, 2 * M], dt.float32, tag="ss")
                    for m in range(M):
                        for t in range(2):
                            junk = junkpool.tile([128, S], dt.bfloat16, tag="junk")
                            nc.scalar.activation(
                                junk[:], z_ps[m][:, 128 * t: 128 * t + S],
                                AF.Square,
                                accum_out=ss[:, 2 * m + t: 2 * m + t + 1])

                    # ---- smalls: mu, var, rs, nb  (all [128, 8]) ----
                    # mean cols ride in z_ps[0][:, 228:236]
                    mu_s = statpool.tile([128, 2 * M], dt.float32, tag="mu_s")
                    nc.vector.tensor_copy(mu_s[:], z_ps[0][:, 228:236])
                    musq = statpool.tile([128, 2 * M], dt.float32, tag="musq")
                    nc.vector.tensor_mul(musq[:], mu_s[:], mu_s[:])
                    var = statpool.tile([128, 2 * M], dt.float32, tag="var")
                    nc.vector.scalar_tensor_tensor(
                        var[:], ss[:], 1.0 / H, musq[:],
                        op0=OP.mult, op1=OP.subtract)
                    varc = statpool.tile([128, 2 * M], dt.float32, tag="varc")
                    nc.vector.tensor_scalar(
                        varc[:], var[:], 0.0, None, op0=OP.max)
                    sd = statpool.tile([128, 2 * M], dt.float32, tag="sd")
                    nc.scalar.activation(sd[:], varc[:], AF.Sqrt, bias=eps_sb[:])
                    rs = statpool.tile([128, 2 * M], dt.float32, tag="rs")
                    nc.vector.reciprocal(rs[:], sd[:])
                    nb = statpool.tile([128, 2 * M], dt.float32, tag="nb")
                    nc.vector.scalar_tensor_tensor(
                        nb[:], mu_s[:], -1.0, rs[:], op0=OP.mult, op1=OP.mult)

                    # ---- per-mask epilogue ----
                    acc = accpool.tile([128, H], dt.float32, tag="acc")
                    for m in range(M):
                        h1 = h_ps[m][:, 0:H]
                        h2 = h_ps[m][:, H:2 * H]
                        c1 = slice(2 * m, 2 * m + 1)
                        c2 = slice(2 * m + 1, 2 * m + 2)
                        # s = sigmoid(rs1*h1 - mu1*rs1)
                        s_sb = spool.tile([128, H], dt.float32, tag="s")
                        nc.scalar.activation(
                            s_sb[:], h1, AF.Sigmoid,
                            bias=nb[:, c1], scale=rs[:, c1])
                        # t = (h2 - mu2) * s
                        t_sb = tpool.tile([128, H], dt.float32, tag="t")
                        nc.vector.scalar_tensor_tensor(
                            t_sb[:], h2, mu_s[:, c2], s_sb[:],
                            op0=OP.subtract, op1=OP.mult)
                        # contribution = max(t, 0) * rs2
                        dst = acc if m == 0 else vpool.tile(
                            [128, H], dt.float32, tag="v")
                        nc.vector.tensor_scalar(
                            dst[:], t_sb[:], 0.0, rs[:, c2],
                            op0=OP.max, op1=OP.mult)
                        if m > 0:
                            acc_eng.tensor_add(acc[:], acc[:], dst[:])

                    nc.sync.dma_start(
                        y_d[bc * 128:(bc + 1) * 128, g * H:(g + 1) * H], acc[:])

    return nc


def _get_state():
    if "nc" not in _STATE:
        _STATE["nc"] = _build_program()
    return _STATE["nc"]


# --------------------------------------------------------------------------
# public entry point
# --------------------------------------------------------------------------

LAST_RESULTS = None


def kernel(x, W_masks, W1, W2, ln1_w, ln1_b, ln2_w, ln2_b):
    global LAST_RESULTS
    import ml_dtypes
    from concourse.bass_utils import run_bass_kernel_spmd

    assert np.allclose(np.asarray(ln1_w), 1.0) and np.allclose(np.asarray(ln2_w), 1.0) \
        and np.allclose(np.asarray(ln1_b), 0.0) and np.allclose(np.asarray(ln2_b), 0.0), \
        "kernel compiled for identity layernorm affine params"

    xt_cores, W_rhs, L_rhs, _MU_rhs = _host_prep(x, W_masks, W1, W2)
    np_dt = {"f32r": np.float32, "f32": np.float32,
             "bf16": ml_dtypes.bfloat16}[MM_DTYPE]
    W_rhs = W_rhs.astype(np_dt)
    L_rhs = L_rhs.astype(np_dt)

    nc = _get_state()
    in_maps = [
        {"xt": xt_cores[c].astype(np_dt), "w": W_rhs, "l": L_rhs}
        for c in range(N_CORES)
    ]
    res = run_bass_kernel_spmd(nc, in_maps, list(range(N_CORES)))
    LAST_RESULTS = res
    out = np.concatenate([res.results[c]["y"] for c in range(N_CORES)], axis=0)
    return out.astype(np.float32)


# revision 12
# speedup vs baseline: 1.4193x; 1.4193x over previous
"""Trainium2 Bass kernel for nn_DynamicFeatureGroupingLayer.

Reference computation (B=4096, G=10 groups of S=100 features, M=4 masks,
H=512 hidden):
    mask = entmax(1.1, W_masks)                       # [G,M,S]
    h_t[b,g,m,:] = (x_g[b] * mask[g,m]) @ W_t[g].T    # t in {1,2}
    n_t = layernorm(h_t)
    out[b,g] = sum_m relu(sigmoid(n_1) * n_2)         # [B, G*H]

Strategy:
  * Data-parallel over batch across 8 cores (512 rows each).
  * Host folds the mask into the weights: W~_t[g,m] = mask[g,m,:] * W_t[g]
    so h_t = x_g @ W~_t.T is a plain matmul (K=S=100, stationary = x chunk).
  * LN means come free as extra matmul columns (sum_h W~/H).
  * LN second moments via a Cholesky-Gram trick: ss = ||L^T x||^2 with
    L = chol(W~^T W~) [S,S]; stats run in x-space (S=100), 4x cheaper than
    h-space (H=512). One big ACT Square pass + one DVE reduce per unit.
  * Sqrt (its own ACT table set) batched per 2-group block so the table
    doesn't thrash against Sigmoid.
  * Epilogue per (group, mask): one ACT Sigmoid with per-partition
    scale/bias (rs1, -mu1*rs1), then a single custom DVE op
    GATE = relu(h2*rs2 + nb2) * s  ==  sigmoid(n1) * relu(n2)
    (valid since s, rs2 > 0); mask-sum accumulation on GPSIMD.
"""

import numpy as np

B = 4096
INPUT_SIZE = 1000
H = 512
M = 4
S = 100
G = 10
N_CORES = 8
BC = B // N_CORES            # batch rows per core (512)
NBC = BC // 128              # 128-row chunks per core (4)
GRP = 2                      # groups g per sqrt-batching block
EPS_LN = 1e-5

# matmul dtype for the PE inputs: "f32r" (full-rate, ~tf32 numerics),
# "f32" (exact, 4x slower), "bf16"
MM_DTYPE = "f32r"
# engine for the mask-sum accumulation: "gpsimd" or "vector"
ACC_ENGINE = "gpsimd"
# how many of the 4 masks take the ACT-Identity path instead of DVE-GATE
J_ACT_PATH = 1

_STATE = {}


# --------------------------------------------------------------------------
# host-side preprocessing
# --------------------------------------------------------------------------

def _entmax(alpha, v):
    v = v - np.max(v, axis=-1, keepdims=True)
    e = np.exp(v)
    s = (np.sum(e ** alpha, axis=-1, keepdims=True) + 1e-5) ** (1.0 / alpha)
    return e / s


def _host_prep(x, W_masks, W1, W2):
    """Returns (xt_per_core, W_rhs, L_rhs) as float32 arrays."""
    x = np.asarray(x, np.float32)
    mask = _entmax(1.1, np.asarray(W_masks, np.float64)).astype(np.float64)
    W1 = np.asarray(W1, np.float64)
    W2 = np.asarray(W2, np.float64)

    # W~_t[g,m,h,s] = mask[g,m,s] * W_t[g,h,s]
    Wt1 = mask[:, :, None, :] * W1[:, None, :, :]        # [G,M,H,S]
    Wt2 = mask[:, :, None, :] * W2[:, None, :, :]
    # main rhs: [G, S, M*2*H], col = m*1024 + t*512 + h
    W_rhs = np.stack([Wt1, Wt2], axis=2)                  # [G,M,2,H,S]
    W_rhs = W_rhs.transpose(0, 4, 1, 2, 3).reshape(G, S, M * 2 * H)

    # mean cols: value = sum_h W~/H; ride in spare tail cols 228:236 of the
    # m=0 block of L_rhs, order 2m+t
    MU = np.stack([Wt1.mean(axis=2), Wt2.mean(axis=2)], axis=2)  # [G,M,2,S]
    MU_rhs = MU.transpose(0, 3, 1, 2).reshape(G, S, 2 * M)

    # cholesky of gram matrices: [G, S, M*256], cols m*256+{0:100 -> L1, 128:228 -> L2}
    L_rhs = np.zeros((G, S, M * 256), np.float64)
    for g in range(G):
        for m in range(M):
            for t, Wt in enumerate((Wt1, Wt2)):
                Wm = Wt[g, m]                              # [H,S]
                Gm = Wm.T @ Wm                             # [S,S]
                jit = 1e-9 * np.trace(Gm) / S
                Lm = np.linalg.cholesky(Gm + jit * np.eye(S))
                L_rhs[g, :, m * 256 + 128 * t: m * 256 + 128 * t + S] = Lm
    L_rhs[:, :, 228:236] = MU_rhs

    # x transposed per core: xt[s, g*512 + b] = x[c*512+b, g*100+s]
    xt_cores = []
    for c in range(N_CORES):
        xc = x[c * BC:(c + 1) * BC]                        # [512, 1000]
        xt = np.ascontiguousarray(
            xc.reshape(BC, G, S).transpose(2, 1, 0).reshape(S, G * BC))
        xt_cores.append(xt)

    return xt_cores, W_rhs.astype(np.float32), L_rhs.astype(np.float32)


# --------------------------------------------------------------------------
# tile patch (this walrus build accepts at most ONE sync wait per inst)
# --------------------------------------------------------------------------

def _install_tile_patch():
    import concourse.mybir as mybir
    from concourse.tile import TileContext, ScopedClock

    if getattr(TileContext, "_drain_patched", False):
        return

    def _patched(self, tick_clock, wait_clock):
        nc = self.nc
        probe = nc.sync.nop(hint="drain_waits", nofuse=True)
        wait_clock.add_sem_waits(
            probe.ins, ScopedClock({None: tick_clock.global_clock}))
        si = probe.ins.sync_info
        if si is not None and len(si.on_wait) > 1:
            waits = list(si.on_wait)
            si.on_wait = [waits[0]]
            probe.ins.sync_info = si
            for w in waits[1:]:
                extra = nc.sync.nop(hint="drain_waits_x", nofuse=True)
                extra.ins.sync_info = mybir.SyncInfo(on_wait=[w], on_update=[])
        nc.sync.drain()
        nc.all_engine_barrier()
        popped = nc._tile_sem_poison_stack.pop()
        assert popped is self._sem_poison
        nc.clear_and_free_semaphores(list(self.sems.allocated().values()))
        nc.all_engine_barrier()

    TileContext._drain_and_barrier = _patched

    # Split extra waits onto dedicated same-engine NOPs committed just
    # before the instruction (sequential blocking on monotonically
    # increasing semaphores is equivalent to a combined wait).
    orig_commit = TileContext._commit_instruction

    def _commit_split(self, inst, lazy_reg_writes=True):
        si = inst.sync_info
        if (
            si is not None
            and len(si.on_wait) > 1
            and inst.engine != mybir.EngineType.Unassigned
        ):
            waits = list(si.on_wait)
            for w in waits[:-1]:
                nop = mybir.InstNoOp(
                    name=self.nc.get_next_instruction_name(),
                    engine=inst.engine,
                    ins=[],
                    outs=[],
                    sync_info=mybir.SyncInfo(on_wait=[w], on_update=[]),
                )
                orig_commit(self, nop, lazy_reg_writes=False)
            si.on_wait = [waits[-1]]
            inst.sync_info = si
        return orig_commit(self, inst, lazy_reg_writes)

    TileContext._commit_instruction = _commit_split
    TileContext._drain_patched = True


# --------------------------------------------------------------------------
# custom DVE op: GATE = relu(Src0*C0 + C1) * Src1
# (= sigmoid(n1) * relu(layernorm(h2)) with C0=rs2, C1=-mu2*rs2, Src1=s)
# --------------------------------------------------------------------------

_GATE_OP = None


def _register_gate_op():
    global _GATE_OP
    if _GATE_OP is not None:
        return _GATE_OP
    import concourse.dve_ops as dve_ops
    from concourse.dve_ops import DveOp, _dve_relu, _CUSTOM_DVE_ROW_BASE
    from concourse.dve_spec import C0, C1, Spec, Src0, Src1, lower, relu
    from concourse.dve_uop import DveOpSpec

    NAME = "TENSOR_GATE_LNRELU"
    for op in dve_ops.OPS:
        if op.name == NAME:
            _GATE_OP = op
            return op

    spec = Spec(
        body=relu(Src0 * C0 + C1) * Src1,
        reference=lambda in0, in1, c0, c1, c2: (
            _dve_relu(in0.astype(np.float32) * c0 + c1) * in1
        ),
    )
    row = _CUSTOM_DVE_ROW_BASE + len(dve_ops.OPS)
    shas = {}
    for ver in ("v3", "v4"):
        try:
            shas[ver] = DveOpSpec(
                name=NAME, opcode=row, uops=lower(spec, ver=ver), rd1_en=True
            ).sha(ver)
        except Exception:
            pass
    op = DveOp(NAME, spec, subdim=False, uops_sha=shas)
    dve_ops.OPS.append(op)
    dve_ops._SUB_OPCODE_FOR_NAME[NAME] = row
    dve_ops.CUSTOM_DVE_SPECS[NAME] = spec
    _GATE_OP = op
    return op


# --------------------------------------------------------------------------
# device kernel
# --------------------------------------------------------------------------

def _build_program():
    import concourse.bass as bass
    import concourse.mybir as mybir
    import concourse.tile as tile

    _install_tile_patch()
    dt = mybir.dt
    AF = mybir.ActivationFunctionType
    OP = mybir.AluOpType
    AX = mybir.AxisListType
    mm_dt = {"f32r": dt.float32r, "f32": dt.float32, "bf16": dt.bfloat16}[MM_DTYPE]
    f16 = dt.float16

    nc = bass.Bass()
    xt_d = nc.declare_dram_parameter("xt", [S, G * BC], mm_dt, isOutput=False)
    w_d = nc.declare_dram_parameter("w", [G, S, M * 2 * H], mm_dt, isOutput=False)
    l_d = nc.declare_dram_parameter("l", [G, S, M * 256], mm_dt, isOutput=False)
    y_d = nc.declare_dram_parameter("y", [BC, G * H], dt.float32, isOutput=True)

    n_blk = G // GRP
    UPB = GRP * NBC          # units per block

    with tile.TileContext(nc) as tc:
        with (
            tc.tile_pool(name="xpool", bufs=1) as xpool,
            tc.tile_pool(name="wpool", bufs=3) as wpool,
            tc.tile_pool(name="lpool", bufs=3) as lpool,
            tc.tile_pool(name="hpsum", bufs=2, space="PSUM") as hpsum,
            tc.tile_pool(name="zpsum", bufs=2, space="PSUM") as zpsum,
            tc.tile_pool(name="ppool", bufs=3) as ppool,
            tc.tile_pool(name="spool", bufs=3) as spool,
            tc.tile_pool(name="npool", bufs=3) as npool,
            tc.tile_pool(name="vpool", bufs=4) as vpool,
            tc.tile_pool(name="accpool", bufs=2) as accpool,
            tc.tile_pool(name="statpool", bufs=2) as statpool,
        ):
            xt_sb = xpool.tile([S, G * BC], mm_dt)
            nc.sync.dma_start(xt_sb[:], xt_d[:])
            eps_sb = xpool.tile([128, 1], dt.float32, tag="eps")
            nc.vector.memset(eps_sb[:], EPS_LN)

            acc_eng = nc.gpsimd if ACC_ENGINE == "gpsimd" else nc.vector

            for blk in range(n_blk):
                gs = [blk * GRP + i for i in range(GRP)]
                w_sbs = {}
                l_sbs = {}
                for g in gs:
                    w_sbs[g] = wpool.tile([S, M * 2 * H], mm_dt, tag="w", name=f"wsb{g}")
                    nc.sync.dma_start(w_sbs[g][:], w_d[g])
                    l_sbs[g] = lpool.tile([S, M * 256], mm_dt, tag="l", name=f"lsb{g}")
                    nc.sync.dma_start(l_sbs[g][:], l_d[g])

                def xch(g, bc):
                    return xt_sb[:, g * BC + bc * 128: g * BC + (bc + 1) * 128]

                # ---- phase A: stats for the whole block ----
                SW = 2 * M * UPB  # stat width (8 cols per unit)
                ss_all = statpool.tile([128, SW], dt.float32, tag="ss")
                mu_all = statpool.tile([128, SW], dt.float32, tag="mu")
                for u, (g, bc) in enumerate((g, bc) for g in gs for bc in range(NBC)):
                    zp = zpsum.tile([128, M * 256], dt.float32, tag="z")
                    for m in range(M):
                        nc.tensor.matmul(
                            zp[:, m * 256:(m + 1) * 256], xch(g, bc),
                            l_sbs[g][:, m * 256:(m + 1) * 256])
                    # squares of the 8 z-blocks (cols m*256+{0:100, 128:228})
                    zv = (zp[:]
                          .rearrange("p (m c) -> p m c", c=256)
                          .rearrange("p m (t r) -> p m t r", r=128)[:, :, :, 0:S])
                    psq = ppool.tile([128, 2 * M * S], f16, tag="p")
                    pv = psq[:].rearrange("p (m t r) -> p m t r", t=2, r=S)
                    nc.scalar.activation(pv, zv, AF.Square)
                    nc.vector.reduce_sum(
                        ss_all[:, u * 2 * M:(u + 1) * 2 * M],
                        psq[:].rearrange("p (q r) -> p q r", r=S),
                        axis=AX.X)
                    nc.vector.tensor_copy(
                        mu_all[:, u * 2 * M:(u + 1) * 2 * M], zp[:, 228:236])

                # ---- block smalls: var, rs, nb ----
                musq = statpool.tile([128, SW], dt.float32, tag="musq")
                nc.vector.tensor_mul(musq[:], mu_all[:], mu_all[:])
                var = statpool.tile([128, SW], dt.float32, tag="var")
                nc.vector.scalar_tensor_tensor(
                    var[:], ss_all[:], 1.0 / H, musq[:],
                    op0=OP.mult, op1=OP.subtract)
                varc = statpool.tile([128, SW], dt.float32, tag="varc")
                nc.vector.tensor_scalar(varc[:], var[:], 0.0, None, op0=OP.max)
                sd = statpool.tile([128, SW], dt.float32, tag="sd")
                nc.scalar.activation(sd[:], varc[:], AF.Sqrt, bias=eps_sb[:])
                rs = statpool.tile([128, SW], dt.float32, tag="rs")
                nc.vector.reciprocal(rs[:], sd[:])
                nb = statpool.tile([128, SW], dt.float32, tag="nb")
                nc.vector.scalar_tensor_tensor(
                    nb[:], mu_all[:], -1.0, rs[:], op0=OP.mult, op1=OP.mult)

                # ---- phase B: main matmuls + epilogue ----
                for u, (g, bc) in enumerate((g, bc) for g in gs for bc in range(NBC)):
                    vs = []
                    for m in range(M):
                        hp = hpsum.tile([128, 2 * H], dt.float32, tag="h")
                        nc.tensor.matmul(
                            hp[:, 0:H], xch(g, bc),
                            w_sbs[g][:, m * 2 * H: m * 2 * H + H])
                        nc.tensor.matmul(
                            hp[:, H:2 * H], xch(g, bc),
                            w_sbs[g][:, m * 2 * H + H:(m + 1) * 2 * H])
                        c1 = slice(u * 2 * M + 2 * m, u * 2 * M + 2 * m + 1)
                        c2 = slice(u * 2 * M + 2 * m + 1, u * 2 * M + 2 * m + 2)
                        s_sb = spool.tile([128, H], f16, tag="s")
                        nc.scalar.activation(
                            s_sb[:], hp[:, 0:H], AF.Sigmoid,
                            bias=nb[:, c1], scale=rs[:, c1])
                        v = vpool.tile([128, H], f16, tag="v", name=f"v{m}")
                        if m < M - J_ACT_PATH:
                            # DVE path: t = (h2 - mu2)*s ; v = max(t,0)*rs2
                            t_sb = npool.tile([128, H], f16, tag="t")
                            nc.vector.scalar_tensor_tensor(
                                t_sb[:], hp[:, H:2 * H], mu_all[:, c2], s_sb[:],
                                op0=OP.subtract, op1=OP.mult)
                            nc.vector.tensor_scalar(
                                v[:], t_sb[:], 0.0, rs[:, c2],
                                op0=OP.max, op1=OP.mult)
                        else:
                            # ACT path: n2 = rs2*h2 + nb2; v = relu(n2)*s
                            n2 = npool.tile([128, H], f16, tag="n2")
                            nc.scalar.activation(
                                n2[:], hp[:, H:2 * H], AF.Identity,
                                bias=nb[:, c2], scale=rs[:, c2])
                            nc.vector.scalar_tensor_tensor(
                                v[:], n2[:], 0.0, s_sb[:],
                                op0=OP.max, op1=OP.mult)
                        vs.append(v)
                    # mask-sum on gpsimd: acc_f32 = (v0+v1) + (v2+v3)
                    w01 = vpool.tile([128, H], f16, tag="w01")
                    acc_eng.tensor_add(w01[:], vs[0][:], vs[1][:])
                    w23 = vpool.tile([128, H], f16, tag="w23")
                    acc_eng.tensor_add(w23[:], vs[2][:], vs[3][:])
                    acc = accpool.tile([128, H], dt.float32, tag="acc")
                    acc_eng.tensor_add(acc[:], w01[:], w23[:])
                    nc.sync.dma_start(
                        y_d[bc * 128:(bc + 1) * 128, g * H:(g + 1) * H], acc[:])

    return nc


def _get_state():
    if "nc" not in _STATE:
        _STATE["nc"] = _build_program()
    return _STATE["nc"]


# --------------------------------------------------------------------------
# public entry point
# --------------------------------------------------------------------------

LAST_RESULTS = None


def kernel(x, W_masks, W1, W2, ln1_w, ln1_b, ln2_w, ln2_b):
    global LAST_RESULTS
    import ml_dtypes
    from concourse.bass_utils import run_bass_kernel_spmd

    assert np.allclose(np.asarray(ln1_w), 1.0) and np.allclose(np.asarray(ln2_w), 1.0) \
        and np.allclose(np.asarray(ln1_b), 0.0) and np.allclose(np.asarray(ln2_b), 0.0), \
        "kernel compiled for identity layernorm affine params"

    xt_cores, W_rhs, L_rhs = _host_prep(x, W_masks, W1, W2)
    np_dt = {"f32r": np.float32, "f32": np.float32,
             "bf16": ml_dtypes.bfloat16}[MM_DTYPE]
    W_rhs = W_rhs.astype(np_dt)
    L_rhs = L_rhs.astype(np_dt)

    nc = _get_state()
    in_maps = [
        {"xt": xt_cores[c].astype(np_dt), "w": W_rhs, "l": L_rhs}
        for c in range(N_CORES)
    ]
    res = run_bass_kernel_spmd(nc, in_maps, list(range(N_CORES)))
    LAST_RESULTS = res
    out = np.concatenate([res.results[c]["y"] for c in range(N_CORES)], axis=0)
    return out.astype(np.float32)


# revision 14
# speedup vs baseline: 1.5618x; 1.1004x over previous
"""Trainium2 Bass kernel for nn_DynamicFeatureGroupingLayer.

Reference computation (B=4096, G=10 groups of S=100 features, M=4 masks,
H=512 hidden):
    mask = entmax(1.1, W_masks)                       # [G,M,S]
    h_t[b,g,m,:] = (x_g[b] * mask[g,m]) @ W_t[g].T    # t in {1,2}
    n_t = layernorm(h_t)
    out[b,g] = sum_m relu(sigmoid(n_1) * n_2)         # [B, G*H]

Strategy:
  * Data-parallel over batch across 8 cores (512 rows each).
  * Host folds the mask into the weights: W~_t[g,m] = mask[g,m,:] * W_t[g]
    so h_t = x_g @ W~_t.T is a plain matmul (K=S=100, stationary = x chunk).
  * LN means come free as extra matmul columns (sum_h W~/H).
  * LN second moments via a Cholesky-Gram trick: ss = ||L^T x||^2 with
    L = chol(W~^T W~) [S,S]; stats run in x-space (S=100), 4x cheaper than
    h-space (H=512). One big ACT Square pass + one DVE reduce per unit.
  * Sqrt (its own ACT table set) batched per 2-group block so the table
    doesn't thrash against Sigmoid.
  * Epilogue per (group, mask): one ACT Sigmoid with per-partition
    scale/bias (rs1, -mu1*rs1), then a single custom DVE op
    GATE = relu(h2*rs2 + nb2) * s  ==  sigmoid(n1) * relu(n2)
    (valid since s, rs2 > 0); mask-sum accumulation on GPSIMD.
"""

import numpy as np

B = 4096
INPUT_SIZE = 1000
H = 512
M = 4
S = 100
G = 10
N_CORES = 8
BC = B // N_CORES            # batch rows per core (512)
NBC = BC // 128              # 128-row chunks per core (4)
GRP = 2                      # groups g per sqrt-batching block
EPS_LN = 1e-5

# matmul dtype for the PE inputs: "f32r" (full-rate, ~tf32 numerics),
# "f32" (exact, 4x slower), "bf16"
MM_DTYPE = "bf16"
# engine for the mask-sum accumulation: "gpsimd" or "vector"
ACC_ENGINE = "gpsimd"
# how many of the 4 masks take the ACT-Identity path instead of DVE-GATE
J_ACT_PATH = 1

_STATE = {}


# --------------------------------------------------------------------------
# host-side preprocessing
# --------------------------------------------------------------------------

def _entmax(alpha, v):
    v = v - np.max(v, axis=-1, keepdims=True)
    e = np.exp(v)
    s = (np.sum(e ** alpha, axis=-1, keepdims=True) + 1e-5) ** (1.0 / alpha)
    return e / s


def _host_prep(x, W_masks, W1, W2):
    """Returns (xt_per_core, W_rhs, L_rhs) as float32 arrays."""
    x = np.asarray(x, np.float32)
    mask = _entmax(1.1, np.asarray(W_masks, np.float64)).astype(np.float64)
    W1 = np.asarray(W1, np.float64)
    W2 = np.asarray(W2, np.float64)

    # W~_t[g,m,h,s] = mask[g,m,s] * W_t[g,h,s]
    Wt1 = mask[:, :, None, :] * W1[:, None, :, :]        # [G,M,H,S]
    Wt2 = mask[:, :, None, :] * W2[:, None, :, :]
    # main rhs: [G, S, M*2*H], col = m*1024 + t*512 + h
    W_rhs = np.stack([Wt1, Wt2], axis=2)                  # [G,M,2,H,S]
    W_rhs = W_rhs.transpose(0, 4, 1, 2, 3).reshape(G, S, M * 2 * H)

    # mean cols: value = sum_h W~/H; ride in spare tail cols 228:236 of the
    # m=0 block of L_rhs, order 2m+t
    MU = np.stack([Wt1.mean(axis=2), Wt2.mean(axis=2)], axis=2)  # [G,M,2,S]
    MU_rhs = MU.transpose(0, 3, 1, 2).reshape(G, S, 2 * M)

    # cholesky of gram matrices: [G, S, M*256], cols m*256+{0:100 -> L1, 128:228 -> L2}
    L_rhs = np.zeros((G, S, M * 256), np.float64)
    for g in range(G):
        for m in range(M):
            for t, Wt in enumerate((Wt1, Wt2)):
                Wm = Wt[g, m]                              # [H,S]
                Gm = Wm.T @ Wm                             # [S,S]
                jit = 1e-9 * np.trace(Gm) / S
                Lm = np.linalg.cholesky(Gm + jit * np.eye(S))
                L_rhs[g, :, m * 256 + 128 * t: m * 256 + 128 * t + S] = Lm
    L_rhs[:, :, 228:236] = MU_rhs

    # x transposed per core: xt[s, g*512 + b] = x[c*512+b, g*100+s]
    xt_cores = []
    for c in range(N_CORES):
        xc = x[c * BC:(c + 1) * BC]                        # [512, 1000]
        xt = np.ascontiguousarray(
            xc.reshape(BC, G, S).transpose(2, 1, 0).reshape(S, G * BC))
        xt_cores.append(xt)

    return xt_cores, W_rhs.astype(np.float32), L_rhs.astype(np.float32)


# --------------------------------------------------------------------------
# tile patch (this walrus build accepts at most ONE sync wait per inst)
# --------------------------------------------------------------------------

def _install_tile_patch():
    import concourse.mybir as mybir
    from concourse.tile import TileContext, ScopedClock

    if getattr(TileContext, "_drain_patched", False):
        return

    def _patched(self, tick_clock, wait_clock):
        nc = self.nc
        probe = nc.sync.nop(hint="drain_waits", nofuse=True)
        wait_clock.add_sem_waits(
            probe.ins, ScopedClock({None: tick_clock.global_clock}))
        si = probe.ins.sync_info
        if si is not None and len(si.on_wait) > 1:
            waits = list(si.on_wait)
            si.on_wait = [waits[0]]
            probe.ins.sync_info = si
            for w in waits[1:]:
                extra = nc.sync.nop(hint="drain_waits_x", nofuse=True)
                extra.ins.sync_info = mybir.SyncInfo(on_wait=[w], on_update=[])
        nc.sync.drain()
        nc.all_engine_barrier()
        popped = nc._tile_sem_poison_stack.pop()
        assert popped is self._sem_poison
        nc.clear_and_free_semaphores(list(self.sems.allocated().values()))
        nc.all_engine_barrier()

    TileContext._drain_and_barrier = _patched

    # Split extra waits onto dedicated same-engine NOPs committed just
    # before the instruction (sequential blocking on monotonically
    # increasing semaphores is equivalent to a combined wait).
    orig_commit = TileContext._commit_instruction

    def _commit_split(self, inst, lazy_reg_writes=True):
        si = inst.sync_info
        if (
            si is not None
            and len(si.on_wait) > 1
            and inst.engine != mybir.EngineType.Unassigned
        ):
            waits = list(si.on_wait)
            for w in waits[:-1]:
                nop = mybir.InstNoOp(
                    name=self.nc.get_next_instruction_name(),
                    engine=inst.engine,
                    ins=[],
                    outs=[],
                    sync_info=mybir.SyncInfo(on_wait=[w], on_update=[]),
                )
                orig_commit(self, nop, lazy_reg_writes=False)
            si.on_wait = [waits[-1]]
            inst.sync_info = si
        return orig_commit(self, inst, lazy_reg_writes)

    TileContext._commit_instruction = _commit_split
    TileContext._drain_patched = True


# --------------------------------------------------------------------------
# custom DVE op: GATE = relu(Src0*C0 + C1) * Src1
# (= sigmoid(n1) * relu(layernorm(h2)) with C0=rs2, C1=-mu2*rs2, Src1=s)
# --------------------------------------------------------------------------

_GATE_OP = None


def _register_gate_op():
    global _GATE_OP
    if _GATE_OP is not None:
        return _GATE_OP
    import concourse.dve_ops as dve_ops
    from concourse.dve_ops import DveOp, _dve_relu, _CUSTOM_DVE_ROW_BASE
    from concourse.dve_spec import C0, C1, Spec, Src0, Src1, lower, relu
    from concourse.dve_uop import DveOpSpec

    NAME = "TENSOR_GATE_LNRELU"
    for op in dve_ops.OPS:
        if op.name == NAME:
            _GATE_OP = op
            return op

    spec = Spec(
        body=relu(Src0 * C0 + C1) * Src1,
        reference=lambda in0, in1, c0, c1, c2: (
            _dve_relu(in0.astype(np.float32) * c0 + c1) * in1
        ),
    )
    row = _CUSTOM_DVE_ROW_BASE + len(dve_ops.OPS)
    shas = {}
    for ver in ("v3", "v4"):
        try:
            shas[ver] = DveOpSpec(
                name=NAME, opcode=row, uops=lower(spec, ver=ver), rd1_en=True
            ).sha(ver)
        except Exception:
            pass
    op = DveOp(NAME, spec, subdim=False, uops_sha=shas)
    dve_ops.OPS.append(op)
    dve_ops._SUB_OPCODE_FOR_NAME[NAME] = row
    dve_ops.CUSTOM_DVE_SPECS[NAME] = spec
    _GATE_OP = op
    return op


# --------------------------------------------------------------------------
# device kernel
# --------------------------------------------------------------------------

def _build_program():
    import concourse.bass as bass
    import concourse.mybir as mybir
    import concourse.tile as tile

    _install_tile_patch()
    dt = mybir.dt
    AF = mybir.ActivationFunctionType
    OP = mybir.AluOpType
    AX = mybir.AxisListType
    mm_dt = {"f32r": dt.float32r, "f32": dt.float32, "bf16": dt.bfloat16}[MM_DTYPE]
    f16 = dt.float16

    nc = bass.Bass()
    xt_d = nc.declare_dram_parameter("xt", [S, G * BC], mm_dt, isOutput=False)
    w_d = nc.declare_dram_parameter("w", [G, S, M * 2 * H], mm_dt, isOutput=False)
    l_d = nc.declare_dram_parameter("l", [G, S, M * 256], mm_dt, isOutput=False)
    y_d = nc.declare_dram_parameter("y", [BC, G * H], dt.float32, isOutput=True)

    n_blk = G // GRP
    UPB = GRP * NBC          # units per block

    with tile.TileContext(nc) as tc:
        with (
            tc.tile_pool(name="xpool", bufs=1) as xpool,
            tc.tile_pool(name="wpool", bufs=3) as wpool,
            tc.tile_pool(name="lpool", bufs=3) as lpool,
            tc.tile_pool(name="hpsum", bufs=2, space="PSUM") as hpsum,
            tc.tile_pool(name="zpsum", bufs=2, space="PSUM") as zpsum,
            tc.tile_pool(name="ppool", bufs=3) as ppool,
            tc.tile_pool(name="spool", bufs=3) as spool,
            tc.tile_pool(name="npool", bufs=3) as npool,
            tc.tile_pool(name="vpool", bufs=4) as vpool,
            tc.tile_pool(name="accpool", bufs=2) as accpool,
            tc.tile_pool(name="statpool", bufs=2) as statpool,
        ):
            xt_sb = xpool.tile([S, G * BC], mm_dt)
            nc.sync.dma_start(xt_sb[:], xt_d[:])
            eps_sb = xpool.tile([128, 1], dt.float32, tag="eps")
            nc.vector.memset(eps_sb[:], EPS_LN)

            acc_eng = nc.gpsimd if ACC_ENGINE == "gpsimd" else nc.vector

            for blk in range(n_blk):
                gs = [blk * GRP + i for i in range(GRP)]
                w_sbs = {}
                l_sbs = {}
                for g in gs:
                    w_sbs[g] = wpool.tile([S, M * 2 * H], mm_dt, tag="w", name=f"wsb{g}")
                    nc.sync.dma_start(w_sbs[g][:], w_d[g])
                    l_sbs[g] = lpool.tile([S, M * 256], mm_dt, tag="l", name=f"lsb{g}")
                    nc.sync.dma_start(l_sbs[g][:], l_d[g])

                def xch(g, bc):
                    return xt_sb[:, g * BC + bc * 128: g * BC + (bc + 1) * 128]

                # ---- phase A: stats for the whole block ----
                SW = 2 * M * UPB  # stat width (8 cols per unit)
                ss_all = statpool.tile([128, SW], dt.float32, tag="ss")
                mu_all = statpool.tile([128, SW], dt.float32, tag="mu")
                for u, (g, bc) in enumerate((g, bc) for g in gs for bc in range(NBC)):
                    zp = zpsum.tile([128, M * 256], dt.float32, tag="z")
                    if MM_DTYPE == "bf16":
                        nc.tensor.matmul(
                            zp[:, 0:512], xch(g, bc), l_sbs[g][:, 0:512])
                        nc.tensor.matmul(
                            zp[:, 512:1024], xch(g, bc), l_sbs[g][:, 512:1024])
                    else:
                        for m in range(M):
                            nc.tensor.matmul(
                                zp[:, m * 256:(m + 1) * 256], xch(g, bc),
                                l_sbs[g][:, m * 256:(m + 1) * 256])
                    # squares of the 8 z-blocks (cols m*256+{0:100, 128:228})
                    zv = (zp[:]
                          .rearrange("p (m c) -> p m c", c=256)
                          .rearrange("p m (t r) -> p m t r", r=128)[:, :, :, 0:S])
                    psq = ppool.tile([128, 2 * M * S], f16, tag="p")
                    pv = psq[:].rearrange("p (m t r) -> p m t r", t=2, r=S)
                    nc.scalar.activation(pv, zv, AF.Square)
                    nc.vector.reduce_sum(
                        ss_all[:, u * 2 * M:(u + 1) * 2 * M],
                        psq[:].rearrange("p (q r) -> p q r", r=S),
                        axis=AX.X)
                    nc.vector.tensor_copy(
                        mu_all[:, u * 2 * M:(u + 1) * 2 * M], zp[:, 228:236])

                # ---- block smalls: var, rs, nb ----
                musq = statpool.tile([128, SW], dt.float32, tag="musq")
                nc.vector.tensor_mul(musq[:], mu_all[:], mu_all[:])
                var = statpool.tile([128, SW], dt.float32, tag="var")
                nc.vector.scalar_tensor_tensor(
                    var[:], ss_all[:], 1.0 / H, musq[:],
                    op0=OP.mult, op1=OP.subtract)
                varc = statpool.tile([128, SW], dt.float32, tag="varc")
                nc.vector.tensor_scalar(varc[:], var[:], 0.0, None, op0=OP.max)
                sd = statpool.tile([128, SW], dt.float32, tag="sd")
                nc.scalar.activation(sd[:], varc[:], AF.Sqrt, bias=eps_sb[:])
                rs = statpool.tile([128, SW], dt.float32, tag="rs")
                nc.vector.reciprocal(rs[:], sd[:])
                nb = statpool.tile([128, SW], dt.float32, tag="nb")
                nc.vector.scalar_tensor_tensor(
                    nb[:], mu_all[:], -1.0, rs[:], op0=OP.mult, op1=OP.mult)

                # ---- phase B: main matmuls + epilogue ----
                for u, (g, bc) in enumerate((g, bc) for g in gs for bc in range(NBC)):
                    vs = []
                    for m in range(M):
                        hp = hpsum.tile([128, 2 * H], dt.float32, tag="h")
                        nc.tensor.matmul(
                            hp[:, 0:H], xch(g, bc),
                            w_sbs[g][:, m * 2 * H: m * 2 * H + H])
                        nc.tensor.matmul(
                            hp[:, H:2 * H], xch(g, bc),
                            w_sbs[g][:, m * 2 * H + H:(m + 1) * 2 * H])
                        c1 = slice(u * 2 * M + 2 * m, u * 2 * M + 2 * m + 1)
                        c2 = slice(u * 2 * M + 2 * m + 1, u * 2 * M + 2 * m + 2)
                        s_sb = spool.tile([128, H], f16, tag="s")
                        nc.scalar.activation(
                            s_sb[:], hp[:, 0:H], AF.Sigmoid,
                            bias=nb[:, c1], scale=rs[:, c1])
                        v = vpool.tile([128, H], f16, tag="v", name=f"v{m}")
                        if m < M - J_ACT_PATH:
                            # DVE path: t = (h2 - mu2)*s ; v = max(t,0)*rs2
                            t_sb = npool.tile([128, H], f16, tag="t")
                            nc.vector.scalar_tensor_tensor(
                                t_sb[:], hp[:, H:2 * H], mu_all[:, c2], s_sb[:],
                                op0=OP.subtract, op1=OP.mult)
                            nc.vector.tensor_scalar(
                                v[:], t_sb[:], 0.0, rs[:, c2],
                                op0=OP.max, op1=OP.mult)
                        else:
                            # ACT path: n2 = rs2*h2 + nb2; v = relu(n2)*s
                            n2 = npool.tile([128, H], f16, tag="n2")
                            nc.scalar.activation(
                                n2[:], hp[:, H:2 * H], AF.Identity,
                                bias=nb[:, c2], scale=rs[:, c2])
                            nc.vector.scalar_tensor_tensor(
                                v[:], n2[:], 0.0, s_sb[:],
                                op0=OP.max, op1=OP.mult)
                        vs.append(v)
                    # mask-sum on gpsimd: acc_f32 = (v0+v1) + (v2+v3)
                    w01 = vpool.tile([128, H], f16, tag="w01")
                    acc_eng.tensor_add(w01[:], vs[0][:], vs[1][:])
                    w23 = vpool.tile([128, H], f16, tag="w23")
                    acc_eng.tensor_add(w23[:], vs[2][:], vs[3][:])
                    acc = accpool.tile([128, H], dt.float32, tag="acc")
                    acc_eng.tensor_add(acc[:], w01[:], w23[:])
                    nc.sync.dma_start(
                        y_d[bc * 128:(bc + 1) * 128, g * H:(g + 1) * H], acc[:])

    return nc


def _get_state():
    if "nc" not in _STATE:
        _STATE["nc"] = _build_program()
    return _STATE["nc"]


# --------------------------------------------------------------------------
# public entry point
# --------------------------------------------------------------------------

LAST_RESULTS = None


def kernel(x, W_masks, W1, W2, ln1_w, ln1_b, ln2_w, ln2_b):
    global LAST_RESULTS
    import ml_dtypes
    from concourse.bass_utils import run_bass_kernel_spmd

    assert np.allclose(np.asarray(ln1_w), 1.0) and np.allclose(np.asarray(ln2_w), 1.0) \
        and np.allclose(np.asarray(ln1_b), 0.0) and np.allclose(np.asarray(ln2_b), 0.0), \
        "kernel compiled for identity layernorm affine params"

    xt_cores, W_rhs, L_rhs = _host_prep(x, W_masks, W1, W2)
    np_dt = {"f32r": np.float32, "f32": np.float32,
             "bf16": ml_dtypes.bfloat16}[MM_DTYPE]
    W_rhs = W_rhs.astype(np_dt)
    L_rhs = L_rhs.astype(np_dt)

    nc = _get_state()
    in_maps = [
        {"xt": xt_cores[c].astype(np_dt), "w": W_rhs, "l": L_rhs}
        for c in range(N_CORES)
    ]
    res = run_bass_kernel_spmd(nc, in_maps, list(range(N_CORES)))
    LAST_RESULTS = res
    out = np.concatenate([res.results[c]["y"] for c in range(N_CORES)], axis=0)
    return out.astype(np.float32)


# revision 18
# speedup vs baseline: 1.5978x; 1.0231x over previous
"""Trainium2 Bass kernel for nn_DynamicFeatureGroupingLayer.

Reference computation (B=4096, G=10 groups of S=100 features, M=4 masks,
H=512 hidden):
    mask = entmax(1.1, W_masks)                       # [G,M,S]
    h_t[b,g,m,:] = (x_g[b] * mask[g,m]) @ W_t[g].T    # t in {1,2}
    n_t = layernorm(h_t)
    out[b,g] = sum_m relu(sigmoid(n_1) * n_2)         # [B, G*H]

Strategy:
  * Data-parallel over batch across 8 cores (512 rows each).
  * Host folds the mask into the weights: W~_t[g,m] = mask[g,m,:] * W_t[g]
    so h_t = x_g @ W~_t.T is a plain matmul (K=S=100, stationary = x chunk).
  * LN means come free as extra matmul columns (sum_h W~/H).
  * LN second moments via a Cholesky-Gram trick: ss = ||L^T x||^2 with
    L = chol(W~^T W~) [S,S]; stats run in x-space (S=100), 4x cheaper than
    h-space (H=512). One big ACT Square pass + one DVE reduce per unit.
  * Sqrt (its own ACT table set) batched per 2-group block so the table
    doesn't thrash against Sigmoid.
  * Epilogue per (group, mask): one ACT Sigmoid with per-partition
    scale/bias (rs1, -mu1*rs1), then a single custom DVE op
    GATE = relu(h2*rs2 + nb2) * s  ==  sigmoid(n1) * relu(n2)
    (valid since s, rs2 > 0); mask-sum accumulation on GPSIMD.
"""

import numpy as np

B = 4096
INPUT_SIZE = 1000
H = 512
M = 4
S = 100
G = 10
N_CORES = 8
BC = B // N_CORES            # batch rows per core (512)
NBC = BC // 128              # 128-row chunks per core (4)
GRP = 2                      # groups g per sqrt-batching block
EPS_LN = 1e-5

# matmul dtype for the PE inputs: "f32r" (full-rate, ~tf32 numerics),
# "f32" (exact, 4x slower), "bf16"
MM_DTYPE = "bf16"
# engine for the mask-sum accumulation: "gpsimd" or "vector"
ACC_ENGINE = "gpsimd"
# how many of the 4 masks take the ACT-Identity path instead of DVE-GATE
J_ACT_PATH = 1

_STATE = {}


# --------------------------------------------------------------------------
# host-side preprocessing
# --------------------------------------------------------------------------

def _entmax(alpha, v):
    v = v - np.max(v, axis=-1, keepdims=True)
    e = np.exp(v)
    s = (np.sum(e ** alpha, axis=-1, keepdims=True) + 1e-5) ** (1.0 / alpha)
    return e / s


def _host_prep(x, W_masks, W1, W2):
    """Returns (xt_per_core, W_rhs, L_rhs) as float32 arrays."""
    x = np.asarray(x, np.float32)
    mask = _entmax(1.1, np.asarray(W_masks, np.float64)).astype(np.float64)
    W1 = np.asarray(W1, np.float64)
    W2 = np.asarray(W2, np.float64)

    # W~_t[g,m,h,s] = mask[g,m,s] * W_t[g,h,s]
    Wt1 = mask[:, :, None, :] * W1[:, None, :, :]        # [G,M,H,S]
    Wt2 = mask[:, :, None, :] * W2[:, None, :, :]
    # main rhs: [G, S, M*2*H], col = m*1024 + t*512 + h
    W_rhs = np.stack([Wt1, Wt2], axis=2)                  # [G,M,2,H,S]
    W_rhs = W_rhs.transpose(0, 4, 1, 2, 3).reshape(G, S, M * 2 * H)

    # mean cols: value = sum_h W~/H; ride in spare tail cols 228:236 of the
    # m=0 block of L_rhs, order 2m+t
    MU = np.stack([Wt1.mean(axis=2), Wt2.mean(axis=2)], axis=2)  # [G,M,2,S]
    MU_rhs = MU.transpose(0, 3, 1, 2).reshape(G, S, 2 * M)

    # cholesky of gram matrices: [G, S, M*256], cols m*256+{0:100 -> L1, 128:228 -> L2}
    L_rhs = np.zeros((G, S, M * 256), np.float64)
    for g in range(G):
        for m in range(M):
            for t, Wt in enumerate((Wt1, Wt2)):
                Wm = Wt[g, m]                              # [H,S]
                Gm = Wm.T @ Wm                             # [S,S]
                jit = 1e-9 * np.trace(Gm) / S
                Lm = np.linalg.cholesky(Gm + jit * np.eye(S))
                L_rhs[g, :, m * 256 + 128 * t: m * 256 + 128 * t + S] = Lm
    L_rhs[:, :, 228:236] = MU_rhs

    # x transposed per core: xt[s, g*512 + b] = x[c*512+b, g*100+s]
    xt_cores = []
    for c in range(N_CORES):
        xc = x[c * BC:(c + 1) * BC]                        # [512, 1000]
        xt = np.ascontiguousarray(
            xc.reshape(BC, G, S).transpose(2, 1, 0).reshape(S, G * BC))
        xt_cores.append(xt)

    return xt_cores, W_rhs.astype(np.float32), L_rhs.astype(np.float32)


# --------------------------------------------------------------------------
# tile patch (this walrus build accepts at most ONE sync wait per inst)
# --------------------------------------------------------------------------

def _install_tile_patch():
    import concourse.mybir as mybir
    from concourse.tile import TileContext, ScopedClock

    if getattr(TileContext, "_drain_patched", False):
        return

    def _patched(self, tick_clock, wait_clock):
        nc = self.nc
        probe = nc.sync.nop(hint="drain_waits", nofuse=True)
        wait_clock.add_sem_waits(
            probe.ins, ScopedClock({None: tick_clock.global_clock}))
        si = probe.ins.sync_info
        if si is not None and len(si.on_wait) > 1:
            waits = list(si.on_wait)
            si.on_wait = [waits[0]]
            probe.ins.sync_info = si
            for w in waits[1:]:
                extra = nc.sync.nop(hint="drain_waits_x", nofuse=True)
                extra.ins.sync_info = mybir.SyncInfo(on_wait=[w], on_update=[])
        nc.sync.drain()
        nc.all_engine_barrier()
        popped = nc._tile_sem_poison_stack.pop()
        assert popped is self._sem_poison
        nc.clear_and_free_semaphores(list(self.sems.allocated().values()))
        nc.all_engine_barrier()

    TileContext._drain_and_barrier = _patched

    # Split extra waits onto dedicated same-engine NOPs committed just
    # before the instruction (sequential blocking on monotonically
    # increasing semaphores is equivalent to a combined wait).
    orig_commit = TileContext._commit_instruction

    def _commit_split(self, inst, lazy_reg_writes=True):
        si = inst.sync_info
        if (
            si is not None
            and len(si.on_wait) > 1
            and inst.engine != mybir.EngineType.Unassigned
        ):
            waits = list(si.on_wait)
            for w in waits[:-1]:
                nop = mybir.InstNoOp(
                    name=self.nc.get_next_instruction_name(),
                    engine=inst.engine,
                    ins=[],
                    outs=[],
                    sync_info=mybir.SyncInfo(on_wait=[w], on_update=[]),
                )
                orig_commit(self, nop, lazy_reg_writes=False)
            si.on_wait = [waits[-1]]
            inst.sync_info = si
        return orig_commit(self, inst, lazy_reg_writes)

    TileContext._commit_instruction = _commit_split
    TileContext._drain_patched = True



# --------------------------------------------------------------------------
# custom DVE op: GATE = relu(Src0*C0 + C1) * Src1
# (= sigmoid(n1) * relu(layernorm(h2)) with C0=rs2, C1=-mu2*rs2, Src1=s)
# --------------------------------------------------------------------------

_GATE_OP = None


def _register_gate_op():
    global _GATE_OP
    if _GATE_OP is not None:
        return _GATE_OP
    import concourse.dve_ops as dve_ops
    from concourse.dve_ops import DveOp, _dve_relu, _CUSTOM_DVE_ROW_BASE
    from concourse.dve_spec import C0, C1, Spec, Src0, Src1, lower, relu
    from concourse.dve_uop import DveOpSpec

    NAME = "TENSOR_GATE_LNRELU"
    for op in dve_ops.OPS:
        if op.name == NAME:
            _GATE_OP = op
            return op

    spec = Spec(
        body=relu(Src0 * C0 + C1) * Src1,
        reference=lambda in0, in1, c0, c1, c2: (
            _dve_relu(in0.astype(np.float32) * c0 + c1) * in1
        ),
    )
    row = _CUSTOM_DVE_ROW_BASE + len(dve_ops.OPS)
    shas = {}
    for ver in ("v3", "v4"):
        try:
            shas[ver] = DveOpSpec(
                name=NAME, opcode=row, uops=lower(spec, ver=ver), rd1_en=True
            ).sha(ver)
        except Exception:
            pass
    op = DveOp(NAME, spec, subdim=False, uops_sha=shas)
    dve_ops.OPS.append(op)
    dve_ops._SUB_OPCODE_FOR_NAME[NAME] = row
    dve_ops.CUSTOM_DVE_SPECS[NAME] = spec
    _GATE_OP = op
    return op


# --------------------------------------------------------------------------
# device kernel
# --------------------------------------------------------------------------

def _build_program():
    import concourse.bass as bass
    import concourse.mybir as mybir
    import concourse.tile as tile

    _install_tile_patch()
    dt = mybir.dt
    AF = mybir.ActivationFunctionType
    OP = mybir.AluOpType
    AX = mybir.AxisListType
    mm_dt = {"f32r": dt.float32r, "f32": dt.float32, "bf16": dt.bfloat16}[MM_DTYPE]
    f16 = dt.bfloat16

    nc = bass.Bass()
    xt_d = nc.declare_dram_parameter("xt", [S, G * BC], mm_dt, isOutput=False)
    w_d = nc.declare_dram_parameter("w", [G, S, M * 2 * H], mm_dt, isOutput=False)
    l_d = nc.declare_dram_parameter("l", [G, S, M * 256], mm_dt, isOutput=False)
    y_d = nc.declare_dram_parameter("y", [BC, G * H], dt.float32, isOutput=True)

    n_blk = G // GRP
    UPB = GRP * NBC          # units per block

    with tile.TileContext(nc) as tc:
        with (
            tc.tile_pool(name="xpool", bufs=1) as xpool,
            tc.tile_pool(name="wpool", bufs=3) as wpool,
            tc.tile_pool(name="lpool", bufs=3) as lpool,
            tc.tile_pool(name="hpsum", bufs=3, space="PSUM") as hpsum,
            tc.tile_pool(name="zpsum", bufs=1, space="PSUM") as zpsum,
            tc.tile_pool(name="ppool", bufs=3) as ppool,
            tc.tile_pool(name="spool", bufs=3) as spool,
            tc.tile_pool(name="npool", bufs=3) as npool,
            tc.tile_pool(name="vpool", bufs=4) as vpool,
            tc.tile_pool(name="accpool", bufs=2) as accpool,
            tc.tile_pool(name="statpool", bufs=2) as statpool,
        ):
            xt_sb = xpool.tile([S, G * BC], mm_dt)
            nc.sync.dma_start(xt_sb[:], xt_d[:])
            eps_sb = xpool.tile([128, 1], dt.float32, tag="eps")
            nc.vector.memset(eps_sb[:], EPS_LN)

            acc_eng = nc.gpsimd if ACC_ENGINE == "gpsimd" else nc.vector

            for blk in range(n_blk):
                gs = [blk * GRP + i for i in range(GRP)]
                w_sbs = {}
                l_sbs = {}
                for g in gs:
                    w_sbs[g] = wpool.tile([S, M * 2 * H], mm_dt, tag="w", name=f"wsb{g}")
                    nc.sync.dma_start(w_sbs[g][:], w_d[g])
                    l_sbs[g] = lpool.tile([S, M * 256], mm_dt, tag="l", name=f"lsb{g}")
                    nc.sync.dma_start(l_sbs[g][:], l_d[g])

                def xch(g, bc):
                    return xt_sb[:, g * BC + bc * 128: g * BC + (bc + 1) * 128]

                # ---- phase A: stats for the whole block ----
                SW = 2 * M * UPB  # stat width (8 cols per unit)
                ss_all = statpool.tile([128, SW], dt.float32, tag="ss")
                mu_all = statpool.tile([128, SW], dt.float32, tag="mu")
                for u, (g, bc) in enumerate((g, bc) for g in gs for bc in range(NBC)):
                    za = zpsum.tile([128, 512], dt.float32, tag="za")
                    zb = zpsum.tile([128, 512], dt.float32, tag="zb")
                    nc.tensor.matmul(za[:], xch(g, bc), l_sbs[g][:, 0:512])
                    nc.tensor.matmul(zb[:], xch(g, bc), l_sbs[g][:, 512:1024])
                    # squares of the 8 z-blocks (cols m*256+{0:100, 128:228})
                    psq = ppool.tile([128, 2 * M * S], f16, tag="p")
                    for half, zt in ((0, za), (1, zb)):
                        zv = (zt[:]
                              .rearrange("p (m c) -> p m c", c=256)
                              .rearrange("p m (t r) -> p m t r", r=128)[:, :, :, 0:S])
                        pv = (psq[:, half * 4 * S:(half + 1) * 4 * S]
                              .rearrange("p (m t r) -> p m t r", t=2, r=S))
                        nc.scalar.activation(pv, zv, AF.Square)
                    nc.vector.reduce_sum(
                        ss_all[:, u * 2 * M:(u + 1) * 2 * M],
                        psq[:].rearrange("p (q r) -> p q r", r=S),
                        axis=AX.X)
                    nc.vector.tensor_copy(
                        mu_all[:, u * 2 * M:(u + 1) * 2 * M], za[:, 228:236])

                # ---- block smalls: var, rs, nb ----
                musq = statpool.tile([128, SW], dt.float32, tag="musq")
                nc.vector.tensor_mul(musq[:], mu_all[:], mu_all[:])
                var = statpool.tile([128, SW], dt.float32, tag="var")
                nc.vector.scalar_tensor_tensor(
                    var[:], ss_all[:], 1.0 / H, musq[:],
                    op0=OP.mult, op1=OP.subtract)
                varc = statpool.tile([128, SW], dt.float32, tag="varc")
                nc.vector.tensor_scalar(varc[:], var[:], 0.0, None, op0=OP.max)
                sd = statpool.tile([128, SW], dt.float32, tag="sd")
                nc.scalar.activation(sd[:], varc[:], AF.Sqrt, bias=eps_sb[:])
                rs = statpool.tile([128, SW], dt.float32, tag="rs")
                nc.vector.reciprocal(rs[:], sd[:])
                nb = statpool.tile([128, SW], dt.float32, tag="nb")
                nc.vector.scalar_tensor_tensor(
                    nb[:], mu_all[:], -1.0, rs[:], op0=OP.mult, op1=OP.mult)

                # ---- phase B: main matmuls + epilogue ----
                for u, (g, bc) in enumerate((g, bc) for g in gs for bc in range(NBC)):
                    vs = []
                    for m in range(M):
                        hp = hpsum.tile([128, 2 * H], dt.float32, tag="h")
                        nc.tensor.matmul(
                            hp[:, 0:H], xch(g, bc),
                            w_sbs[g][:, m * 2 * H: m * 2 * H + H])
                        nc.tensor.matmul(
                            hp[:, H:2 * H], xch(g, bc),
                            w_sbs[g][:, m * 2 * H + H:(m + 1) * 2 * H])
                        c1 = slice(u * 2 * M + 2 * m, u * 2 * M + 2 * m + 1)
                        c2 = slice(u * 2 * M + 2 * m + 1, u * 2 * M + 2 * m + 2)
                        s_sb = spool.tile([128, H], f16, tag="s")
                        nc.scalar.activation(
                            s_sb[:], hp[:, 0:H], AF.Sigmoid,
                            bias=nb[:, c1], scale=rs[:, c1])
                        v = vpool.tile([128, H], f16, tag="v", name=f"v{m}")
                        if m < M - J_ACT_PATH:
                            # DVE path: t = (h2 - mu2)*s ; v = max(t,0)*rs2
                            t_sb = npool.tile([128, H], f16, tag="t")
                            nc.vector.scalar_tensor_tensor(
                                t_sb[:], hp[:, H:2 * H], mu_all[:, c2], s_sb[:],
                                op0=OP.subtract, op1=OP.mult)
                            nc.vector.tensor_scalar(
                                v[:], t_sb[:], 0.0, rs[:, c2],
                                op0=OP.max, op1=OP.mult)
                        else:
                            # ACT path: n2 = rs2*h2 + nb2; v = relu(n2)*s
                            n2 = npool.tile([128, H], f16, tag="n2")
                            nc.scalar.activation(
                                n2[:], hp[:, H:2 * H], AF.Identity,
                                bias=nb[:, c2], scale=rs[:, c2])
                            nc.vector.scalar_tensor_tensor(
                                v[:], n2[:], 0.0, s_sb[:],
                                op0=OP.max, op1=OP.mult)
                        vs.append(v)
                    # mask-sum on gpsimd: acc_f32 = (v0+v1) + (v2+v3)
                    w01 = vpool.tile([128, H], f16, tag="w01")
                    acc_eng.tensor_add(w01[:], vs[0][:], vs[1][:])
                    w23 = vpool.tile([128, H], f16, tag="w23")
                    acc_eng.tensor_add(w23[:], vs[2][:], vs[3][:])
                    acc = accpool.tile([128, H], dt.float32, tag="acc")
                    acc_eng.tensor_add(acc[:], w01[:], w23[:])
                    nc.sync.dma_start(
                        y_d[bc * 128:(bc + 1) * 128, g * H:(g + 1) * H], acc[:])

    return nc


def _get_state():
    if "nc" not in _STATE:
        _STATE["nc"] = _build_program()
    return _STATE["nc"]


# --------------------------------------------------------------------------
# public entry point
# --------------------------------------------------------------------------

LAST_RESULTS = None


def kernel(x, W_masks, W1, W2, ln1_w, ln1_b, ln2_w, ln2_b):
    global LAST_RESULTS
    import ml_dtypes
    from concourse.bass_utils import run_bass_kernel_spmd

    assert np.allclose(np.asarray(ln1_w), 1.0) and np.allclose(np.asarray(ln2_w), 1.0) \
        and np.allclose(np.asarray(ln1_b), 0.0) and np.allclose(np.asarray(ln2_b), 0.0), \
        "kernel compiled for identity layernorm affine params"

    xt_cores, W_rhs, L_rhs = _host_prep(x, W_masks, W1, W2)
    np_dt = {"f32r": np.float32, "f32": np.float32,
             "bf16": ml_dtypes.bfloat16}[MM_DTYPE]
    W_rhs = W_rhs.astype(np_dt)
    L_rhs = L_rhs.astype(np_dt)

    nc = _get_state()
    in_maps = [
        {"xt": xt_cores[c].astype(np_dt), "w": W_rhs, "l": L_rhs}
        for c in range(N_CORES)
    ]
    res = run_bass_kernel_spmd(nc, in_maps, list(range(N_CORES)))
    LAST_RESULTS = res
    out = np.concatenate([res.results[c]["y"] for c in range(N_CORES)], axis=0)
    return out.astype(np.float32)
